# revision 5
# baseline (speedup 1.0000x reference)
"""Trainium2 Bass kernel for nn_AttentionBlock (biased dense attention).

Math:  x' = x + phi_degree + phi_3d_sum
       S  = (x' Wq)(x' Wk)^T * scaling + phi_spd + phi_edge + phi_3d
       out = softmax(S, axis=-1) @ (x' Wv)

Strategy (8 cores, sequence parallel on q):
  - Host prep: xp = x' (tiny O(n*d) add), A = scaling * Wq @ Wk.T, and
    per-core TRANSPOSED row-shards of the three phi bias matrices
    (phi[qshard, :].T -> [n, n_loc], so the device streams them with
    contiguous 2KB rows directly in the [k, q] orientation).
  - Each core holds x'^T (built via PE identity-matmul transposes) and
    V = x' Wv resident in SBUF, computes B^T = A^T x'_loc^T, then for its
    q-shard computes S^T[k, q] tiles = (x'^T)^T-block @ B^T-chunk + bias.
    Computing S TRANSPOSED means exp(S^T) tiles are directly usable as
    the stationary operand of the P@V matmul - no on-chip transposes of
    the attention matrix at all.
  - Bias = phi_spd + phi_edge + phi_3d is summed for free by chained
    accum-DMA (SDMA inline fp32 adds), added to S^T psum by one DVE op.
  - softmax denominators come for free from a ones-column appended to V.

kernel(**inputs) -> full [8192, 256] fp32 output.
"""

import numpy as np

import concourse.bacc as bacc
import concourse.tile as tile
from concourse import mybir
from concourse.bass_utils import run_bass_kernel_spmd
from concourse.masks import make_identity

N_FULL = 8192
D = 256
CORES = 8
SCALING = 0.0625

f32 = mybir.dt.float32


def build_attention_nc(n, n_loc, d=D, cores=CORES, reps=1):
    """Build the SPMD Bass program (one program, runs on all cores)."""
    assert n % 512 == 0 and n_loc % 128 == 0 and d == 256
    QCH = min(512, n_loc)  # q-chunk: free dim of S^T tiles
    n_qc = n_loc // QCH  # outer q passes
    n_kb = n // 128  # k blocks
    n_db = d // 128  # d blocks (2)
    KSLAB = 4  # k-blocks per phi DMA slab
    ASLAB = 4  # row-blocks per xp load slab
    assert n_kb % KSLAB == 0

    nc = bacc.Bacc("TRN2", target_bir_lowering=False, debug=False, num_devices=cores)
    xp = nc.declare_dram_parameter("xp", [n, d], f32, isOutput=False)
    xp_loc = nc.declare_dram_parameter("xp_loc", [n_loc, d], f32, isOutput=False)
    a_mat = nc.declare_dram_parameter("a_mat", [d, d], f32, isOutput=False)
    wv = nc.declare_dram_parameter("wv", [d, d], f32, isOutput=False)
    phi_names = ("phi_spd_t", "phi_edge_t", "phi_3d_t")
    phis = [
        nc.declare_dram_parameter(nm, [n, n_loc], f32, isOutput=False)
        for nm in phi_names
    ]
    out = nc.declare_dram_parameter("out", [n_loc, d], f32, isOutput=True)

    import contextlib

    with tile.TileContext(nc) as tc:
        loop_ctx = (
            tc.For_i(0, reps, 1) if reps > 1 else contextlib.nullcontext()
        )
        with loop_ctx, tc.tile_pool(name="res", bufs=1) as res:
            ident = res.tile([128, 128], f32)
            make_identity(nc, ident)
            a_sb = res.tile([128, n_db, d], f32)
            nc.sync.dma_start(out=a_sb, in_=a_mat.rearrange("(b p) j -> p b j", p=128))
            wv_sb = res.tile([128, n_db, d], f32)
            nc.sync.dma_start(out=wv_sb, in_=wv.rearrange("(b p) j -> p b j", p=128))

            # resident: x'^T (full), x'_loc^T, B^T, V(+ones)
            xt = [res.tile([128, n], f32, tag=f"xt{b}", name=f"xt{b}") for b in range(n_db)]
            xtl = [res.tile([128, n_loc], f32, tag=f"xtl{b}", name=f"xtl{b}") for b in range(n_db)]
            bt = [res.tile([128, n_loc], f32, tag=f"bt{b}", name=f"bt{b}") for b in range(n_db)]
            v_sb = res.tile([128, n_kb, d + 1], f32)

            # ---------------- Phase A: load xp, transpose to x'^T ----------
            psA_ctx = tc.tile_pool(name="psA", bufs=2, space="PSUM")
            psA = psA_ctx.__enter__()

            def transpose_in(src_dram, nrows, dst_tiles):
                aslab = min(ASLAB, nrows // 128)
                nslab = nrows // (128 * aslab)
                src = src_dram.rearrange("(s a p) d -> s p a d", p=128, a=aslab)
                with tc.tile_pool(name="pha", bufs=2) as pha:
                    for s in range(nslab):
                        slab = pha.tile([128, aslab, d], f32, tag="xpslab")
                        nc.sync.dma_start(out=slab, in_=src[s])
                        for db in range(n_db):
                            pt = psA.tile([128, 128 * aslab], f32, tag="xtp")
                            for a in range(aslab):
                                nc.tensor.matmul(
                                    pt[:, a * 128 : (a + 1) * 128],
                                    slab[:, a, db * 128 : (db + 1) * 128],
                                    ident,
                                    start=True,
                                    stop=True,
                                )
                            nc.scalar.copy(
                                out=dst_tiles[db][
                                    :, s * 128 * aslab : (s + 1) * 128 * aslab
                                ],
                                in_=pt,
                            )

            transpose_in(xp, n, xt)
            transpose_in(xp_loc, n_loc, xtl)

            # ---------------- Phase B: B^T = A^T x'_loc^T ; V = x' Wv ------
            for db2 in range(n_db):
                for q0 in range(0, n_loc, QCH):
                    pb = psA.tile([128, QCH], f32, tag="btp")
                    for d1 in range(n_db):
                        nc.tensor.matmul(
                            pb,
                            a_sb[:, d1, db2 * 128 : (db2 + 1) * 128],
                            xtl[d1][:, q0 : q0 + QCH],
                            start=(d1 == 0),
                            stop=(d1 == n_db - 1),
                        )
                    nc.vector.tensor_copy(bt[db2][:, q0 : q0 + QCH], pb)

            for kb in range(n_kb):
                pv = psA.tile([128, d], f32, tag="vp")
                for d1 in range(n_db):
                    nc.tensor.matmul(
                        pv,
                        xt[d1][:, kb * 128 : (kb + 1) * 128],
                        wv_sb[:, d1, :],
                        start=(d1 == 0),
                        stop=(d1 == n_db - 1),
                    )
                nc.scalar.copy(out=v_sb[:, kb, :d], in_=pv)
            nc.vector.memset(v_sb[:, :, d : d + 1], 1.0)
            psA_ctx.__exit__(None, None, None)

            # ---------------- Phase C: S^T tiles -> exp -> @V --------------
            phi_r = [
                p.rearrange("(sb b p) q -> sb p b q", p=128, b=KSLAB) for p in phis
            ]
            with (
                tc.tile_pool(name="phC", bufs=3) as phC,
                tc.tile_pool(name="sbC", bufs=3) as sbC,
                tc.tile_pool(name="psS", bufs=3, space="PSUM") as psS,
                tc.tile_pool(name="psO", bufs=1, space="PSUM") as psO,
            ):
                for qc in range(n_qc):
                    out_ps = [
                        psO.tile([128, d + 1], f32, tag=f"out{t}", name=f"outp{qc}_{t}")
                        for t in range(QCH // 128)
                    ]
                    phi_slab = None
                    for kb in range(n_kb):
                        if kb % KSLAB == 0:
                            sb_i = kb // KSLAB
                            phi_slab = phC.tile([128, KSLAB, QCH], f32, tag="phisum")
                            for i, pr in enumerate(phi_r):
                                nc.gpsimd.dma_start(
                                    out=phi_slab,
                                    in_=pr[sb_i][:, :, qc * QCH : (qc + 1) * QCH],
                                    accum_op=(
                                        mybir.AluOpType.bypass
                                        if i == 0
                                        else mybir.AluOpType.add
                                    ),
                                )
                        s_ps = psS.tile([128, QCH], f32, tag="s")
                        for db in range(n_db):
                            nc.tensor.matmul(
                                s_ps,
                                xt[db][:, kb * 128 : (kb + 1) * 128],
                                bt[db][:, qc * QCH : (qc + 1) * QCH],
                                start=(db == 0),
                                stop=(db == n_db - 1),
                            )
                        nc.vector.tensor_add(s_ps, s_ps, phi_slab[:, kb % KSLAB, :])
                        pt = sbC.tile([128, QCH], f32, tag="pt")
                        nc.scalar.activation(
                            out=pt, in_=s_ps, func=mybir.ActivationFunctionType.Exp
                        )
                        for t in range(QCH // 128):
                            nc.tensor.matmul(
                                out_ps[t],
                                pt[:, t * 128 : (t + 1) * 128],
                                v_sb[:, kb, :],
                                start=(kb == 0),
                                stop=(kb == n_kb - 1),
                            )
                    for t in range(QCH // 128):
                        rs = sbC.tile([128, 1], f32, tag="rs")
                        nc.vector.reciprocal(rs, out_ps[t][:, d : d + 1])
                        ob = sbC.tile([128, d], f32, tag="ob")
                        nc.vector.tensor_scalar_mul(ob, out_ps[t][:, :d], rs)
                        r0 = qc * QCH + t * 128
                        nc.sync.dma_start(out=out[r0 : r0 + 128, :], in_=ob)
    nc.compile()
    return nc


def _make_in_maps(xp, A, Wv, phi_spd, phi_edge, phi_3d, n_loc, cores=CORES):
    in_maps = []
    for c in range(cores):
        r0, r1 = c * n_loc, (c + 1) * n_loc
        in_maps.append(
            {
                "xp": xp,
                "xp_loc": np.ascontiguousarray(xp[r0:r1]),
                "a_mat": A,
                "wv": Wv,
                "phi_spd_t": np.ascontiguousarray(phi_spd[r0:r1].T),
                "phi_edge_t": np.ascontiguousarray(phi_edge[r0:r1].T),
                "phi_3d_t": np.ascontiguousarray(phi_3d[r0:r1].T),
            }
        )
    return in_maps


_CACHED_NC = {}


def _get_nc(n, n_loc):
    key = (n, n_loc)
    if key not in _CACHED_NC:
        _CACHED_NC[key] = build_attention_nc(n, n_loc)
    return _CACHED_NC[key]


def kernel(x, phi_degree, phi_3d_sum, phi_3d, phi_spd, phi_edge, Wq, Wk, Wv):
    x = np.asarray(x, dtype=np.float32)
    phi_degree = np.asarray(phi_degree, dtype=np.float32)
    phi_3d_sum = np.asarray(phi_3d_sum, dtype=np.float32)
    phi_3d = np.asarray(phi_3d, dtype=np.float32)
    phi_spd = np.asarray(phi_spd, dtype=np.float32)
    phi_edge = np.asarray(phi_edge, dtype=np.float32)
    Wq = np.asarray(Wq, dtype=np.float32)
    Wk = np.asarray(Wk, dtype=np.float32)
    Wv = np.asarray(Wv, dtype=np.float32)

    n = x.shape[0]
    n_loc = n // CORES
    xp = x + phi_degree + phi_3d_sum
    A = (SCALING * (Wq.astype(np.float64) @ Wk.astype(np.float64).T)).astype(
        np.float32
    )

    nc = _get_nc(n, n_loc)
    in_maps = _make_in_maps(xp, A, Wv, phi_spd, phi_edge, phi_3d, n_loc)
    res = run_bass_kernel_spmd(nc, in_maps, list(range(CORES)))
    return np.concatenate([res.results[c]["out"] for c in range(CORES)], axis=0)


# revision 7
# speedup vs baseline: 1.6029x; 1.6029x over previous
"""Trainium2 Bass kernel for nn_AttentionBlock (biased dense attention).

Math:  x' = x + phi_degree + phi_3d_sum
       S  = (x' Wq)(x' Wk)^T * scaling + phi_spd + phi_edge + phi_3d
       out = softmax(S, axis=-1) @ (x' Wv)

Strategy (8 cores, sequence parallel on q):
  - Host prep: xp = x' (tiny O(n*d) add), A = scaling * Wq @ Wk.T, and
    per-core TRANSPOSED row-shards of the three phi bias matrices
    (phi[qshard, :].T -> [n, n_loc], so the device streams them with
    contiguous 2KB rows directly in the [k, q] orientation).
  - Each core holds x'^T (built via PE identity-matmul transposes) and
    V = x' Wv resident in SBUF, computes B^T = A^T x'_loc^T, then for its
    q-shard computes S^T[k, q] tiles = (x'^T)^T-block @ B^T-chunk + bias.
    Computing S TRANSPOSED means exp(S^T) tiles are directly usable as
    the stationary operand of the P@V matmul - no on-chip transposes of
    the attention matrix at all.
  - Bias = phi_spd + phi_edge + phi_3d is summed for free by chained
    accum-DMA (SDMA inline fp32 adds), added to S^T psum by one DVE op.
  - softmax denominators come for free from a ones-column appended to V.

kernel(**inputs) -> full [8192, 256] fp32 output.
"""

import numpy as np

import concourse.bacc as bacc
import concourse.tile as tile
from concourse import mybir
from concourse.bass_utils import run_bass_kernel_spmd
from concourse.masks import make_identity

N_FULL = 8192
D = 256
CORES = 8
SCALING = 0.0625

f32 = mybir.dt.float32


def build_attention_nc(n, n_loc, d=D, cores=CORES, reps=1, mmdt=f32):
    """Build the SPMD Bass program (one program, runs on all cores)."""
    assert n % 512 == 0 and n_loc % 128 == 0 and d == 256
    QCH = min(512, n_loc)  # q-chunk: free dim of S^T tiles
    n_qc = n_loc // QCH  # outer q passes
    n_kb = n // 128  # k blocks
    n_db = d // 128  # d blocks (2)
    KSLAB = 4  # k-blocks per phi DMA slab
    ASLAB = 4  # row-blocks per xp load slab
    assert n_kb % KSLAB == 0

    vw = d + 1 if mmdt == f32 else d + 2  # fp32r matmul needs even free dim
    nc = bacc.Bacc("TRN2", target_bir_lowering=False, debug=False, num_devices=cores)
    xp = nc.declare_dram_parameter("xp", [n, d], f32, isOutput=False)
    xp_loc = nc.declare_dram_parameter("xp_loc", [n_loc, d], f32, isOutput=False)
    a_mat = nc.declare_dram_parameter("a_mat", [d, d], f32, isOutput=False)
    wv = nc.declare_dram_parameter("wv", [d, d], f32, isOutput=False)
    phi_names = ("phi_spd_t", "phi_edge_t", "phi_3d_t")
    phis = [
        nc.declare_dram_parameter(nm, [n, n_loc], f32, isOutput=False)
        for nm in phi_names
    ]
    out = nc.declare_dram_parameter("out", [n_loc, d], f32, isOutput=True)

    import contextlib

    with tile.TileContext(nc) as tc:
        loop_ctx = (
            tc.For_i(0, reps, 1) if reps > 1 else contextlib.nullcontext()
        )
        with loop_ctx, tc.tile_pool(name="res", bufs=1) as res:
            ident = res.tile([128, 128], f32)
            make_identity(nc, ident)
            a_sb = res.tile([128, n_db, d], mmdt)
            nc.sync.dma_start(out=a_sb, in_=a_mat.rearrange("(b p) j -> p b j", p=128).bitcast(mmdt))
            wv_sb = res.tile([128, n_db, d], mmdt)
            nc.sync.dma_start(out=wv_sb, in_=wv.rearrange("(b p) j -> p b j", p=128).bitcast(mmdt))

            # resident: x'^T (full), x'_loc^T, B^T, V(+ones)
            xt = [res.tile([128, n], mmdt, tag=f"xt{b}", name=f"xt{b}") for b in range(n_db)]
            xtl = [res.tile([128, n_loc], mmdt, tag=f"xtl{b}", name=f"xtl{b}") for b in range(n_db)]
            bt = [res.tile([128, n_loc], mmdt, tag=f"bt{b}", name=f"bt{b}") for b in range(n_db)]
            v_sb = res.tile([128, n_kb, vw], mmdt)

            # ---------------- Phase A: load xp, transpose to x'^T ----------
            psA_ctx = tc.tile_pool(name="psA", bufs=2, space="PSUM")
            psA = psA_ctx.__enter__()

            def transpose_in(src_dram, nrows, dst_tiles):
                aslab = min(ASLAB, nrows // 128)
                nslab = nrows // (128 * aslab)
                src = src_dram.rearrange("(s a p) d -> s p a d", p=128, a=aslab)
                with tc.tile_pool(name="pha", bufs=2) as pha:
                    for s in range(nslab):
                        slab = pha.tile([128, aslab, d], f32, tag="xpslab")
                        nc.sync.dma_start(out=slab, in_=src[s])
                        for db in range(n_db):
                            pt = psA.tile([128, 128 * aslab], f32, tag="xtp")
                            for a in range(aslab):
                                nc.tensor.matmul(
                                    pt[:, a * 128 : (a + 1) * 128],
                                    slab[:, a, db * 128 : (db + 1) * 128],
                                    ident,
                                    start=True,
                                    stop=True,
                                )
                            nc.scalar.copy(
                                out=dst_tiles[db][
                                    :, s * 128 * aslab : (s + 1) * 128 * aslab
                                ],
                                in_=pt,
                            )

            transpose_in(xp, n, xt)
            transpose_in(xp_loc, n_loc, xtl)

            # ---------------- Phase B: B^T = A^T x'_loc^T ; V = x' Wv ------
            for db2 in range(n_db):
                for q0 in range(0, n_loc, QCH):
                    pb = psA.tile([128, QCH], f32, tag="btp")
                    for d1 in range(n_db):
                        nc.tensor.matmul(
                            pb,
                            a_sb[:, d1, db2 * 128 : (db2 + 1) * 128],
                            xtl[d1][:, q0 : q0 + QCH],
                            start=(d1 == 0),
                            stop=(d1 == n_db - 1),
                        )
                    nc.vector.tensor_copy(bt[db2][:, q0 : q0 + QCH], pb)

            for kb in range(n_kb):
                pv = psA.tile([128, d], f32, tag="vp")
                for d1 in range(n_db):
                    nc.tensor.matmul(
                        pv,
                        xt[d1][:, kb * 128 : (kb + 1) * 128],
                        wv_sb[:, d1, :],
                        start=(d1 == 0),
                        stop=(d1 == n_db - 1),
                    )
                nc.scalar.copy(out=v_sb[:, kb, :d], in_=pv)
            nc.vector.memset(v_sb[:, :, d : vw].bitcast(f32), 1.0)
            psA_ctx.__exit__(None, None, None)

            # ---------------- Phase C: S^T tiles -> exp -> @V --------------
            phi_r = [
                p.rearrange("(sb b p) q -> sb p b q", p=128, b=KSLAB) for p in phis
            ]
            with (
                tc.tile_pool(name="phC", bufs=3) as phC,
                tc.tile_pool(name="sbC", bufs=3) as sbC,
                tc.tile_pool(name="psS", bufs=3, space="PSUM") as psS,
                tc.tile_pool(name="psO", bufs=1, space="PSUM") as psO,
            ):
                for qc in range(n_qc):
                    out_ps = [
                        psO.tile([128, vw], f32, tag=f"out{t}", name=f"outp{qc}_{t}")
                        for t in range(QCH // 128)
                    ]
                    phi_slab = None
                    for kb in range(n_kb):
                        if kb % KSLAB == 0:
                            sb_i = kb // KSLAB
                            phi_slab = phC.tile([128, KSLAB, QCH], f32, tag="phisum")
                            for i, pr in enumerate(phi_r):
                                nc.gpsimd.dma_start(
                                    out=phi_slab,
                                    in_=pr[sb_i][:, :, qc * QCH : (qc + 1) * QCH],
                                    accum_op=(
                                        mybir.AluOpType.bypass
                                        if i == 0
                                        else mybir.AluOpType.add
                                    ),
                                )
                        s_ps = psS.tile([128, QCH], f32, tag="s")
                        for db in range(n_db):
                            nc.tensor.matmul(
                                s_ps,
                                xt[db][:, kb * 128 : (kb + 1) * 128],
                                bt[db][:, qc * QCH : (qc + 1) * QCH],
                                start=(db == 0),
                                stop=(db == n_db - 1),
                            )
                        nc.vector.tensor_add(s_ps, s_ps, phi_slab[:, kb % KSLAB, :])
                        pt = sbC.tile([128, QCH], mmdt, tag="pt")
                        nc.scalar.activation(
                            out=pt, in_=s_ps, func=mybir.ActivationFunctionType.Exp
                        )
                        for t in range(QCH // 128):
                            nc.tensor.matmul(
                                out_ps[t],
                                pt[:, t * 128 : (t + 1) * 128],
                                v_sb[:, kb, :],
                                start=(kb == 0),
                                stop=(kb == n_kb - 1),
                            )
                    for t in range(QCH // 128):
                        rs = sbC.tile([128, 1], f32, tag="rs")
                        nc.vector.reciprocal(rs, out_ps[t][:, d : d + 1])
                        ob = sbC.tile([128, d], f32, tag="ob")
                        nc.vector.tensor_scalar_mul(ob, out_ps[t][:, :d], rs)
                        r0 = qc * QCH + t * 128
                        nc.sync.dma_start(out=out[r0 : r0 + 128, :], in_=ob)
    nc.compile()
    return nc


def _make_in_maps(xp, A, Wv, phi_spd, phi_edge, phi_3d, n_loc, cores=CORES):
    in_maps = []
    for c in range(cores):
        r0, r1 = c * n_loc, (c + 1) * n_loc
        in_maps.append(
            {
                "xp": xp,
                "xp_loc": np.ascontiguousarray(xp[r0:r1]),
                "a_mat": A,
                "wv": Wv,
                "phi_spd_t": np.ascontiguousarray(phi_spd[r0:r1].T),
                "phi_edge_t": np.ascontiguousarray(phi_edge[r0:r1].T),
                "phi_3d_t": np.ascontiguousarray(phi_3d[r0:r1].T),
            }
        )
    return in_maps


_CACHED_NC = {}


def _get_nc(n, n_loc):
    key = (n, n_loc)
    if key not in _CACHED_NC:
        _CACHED_NC[key] = build_attention_nc(n, n_loc)
    return _CACHED_NC[key]


def kernel(x, phi_degree, phi_3d_sum, phi_3d, phi_spd, phi_edge, Wq, Wk, Wv):
    x = np.asarray(x, dtype=np.float32)
    phi_degree = np.asarray(phi_degree, dtype=np.float32)
    phi_3d_sum = np.asarray(phi_3d_sum, dtype=np.float32)
    phi_3d = np.asarray(phi_3d, dtype=np.float32)
    phi_spd = np.asarray(phi_spd, dtype=np.float32)
    phi_edge = np.asarray(phi_edge, dtype=np.float32)
    Wq = np.asarray(Wq, dtype=np.float32)
    Wk = np.asarray(Wk, dtype=np.float32)
    Wv = np.asarray(Wv, dtype=np.float32)

    n = x.shape[0]
    n_loc = n // CORES
    xp = x + phi_degree + phi_3d_sum
    A = (SCALING * (Wq.astype(np.float64) @ Wk.astype(np.float64).T)).astype(
        np.float32
    )

    nc = _get_nc(n, n_loc)
    in_maps = _make_in_maps(xp, A, Wv, phi_spd, phi_edge, phi_3d, n_loc)
    res = run_bass_kernel_spmd(nc, in_maps, list(range(CORES)))
    return np.concatenate([res.results[c]["out"] for c in range(CORES)], axis=0)
